# revision 1
# baseline (speedup 1.0000x reference)
"""Trainium2 Bass kernel v2 for nn_MultiHeadAttention_74491912782210.

fp16-X redesign: X streams from HBM as fp16 (64 MB/core vs 128 fp32),
halving the dominant DMA cost. Score precision is preserved by splitting
the rank-16 projection wkq = blockdiag(q)-proj of Wk into fp16 hi + lo
parts accumulated in the same fp32 PSUM group (measured end-to-end rel
err 1.2e-2 vs the 2e-2 gate, numpy-simulated on the exact inputs).

Per batch (two passes over the SBUF-resident 8 MB fp16 batch):
  pass 1 (per 512-row chunk): PE-transpose X -> xt, then col-tiled score
    matmuls: 4 PE column groups each own a 128-row s-slice, accumulating
    8 m-tiles x {hi,lo} into PSUM partitions 32j..32j+16. No cross-group
    combines needed; one evac per chunk into scT [128, nch*128].
  softmax (batched): free-dim max-reduce -> PE transpose to a [1,128]
    row -> combine the 4 j-blocks in the free dim -> PE transpose back
    to a per-partition bias; one ACT exp over [128, nch*128] with
    accum_out giving l; same row trick for 1/l; p scaled in place.
  pass 2: col-tiled ctx matmuls (group j owns s-subtile j of each
    chunk), 4x concurrent, accumulated across the whole batch in PSUM;
    combine the 4 group partials once per batch; transpose into ctxT.

Endgame: ho = ctxT^T @ Wv for all (h,b) at once (diag blocks extracted
via PE transposes), fc via fcT, LayerNorm. Weights Wv/fc are fp16.

Sharding: batch-parallel, 8 batches per core x 8 cores. No collectives.
"""
import os
import numpy as np
from contextlib import ExitStack

import concourse.bacc as bacc
import concourse.tile as tile
import concourse.mybir as mybir
from concourse import bass_utils

f32 = mybir.dt.float32
f16 = mybir.dt.float16
AF = mybir.ActivationFunctionType
OP = mybir.AluOpType
AX = mybir.AxisListType

D = 1024
H = 16
DH = 64
CS = 512            # rows per chunk
MT = D // 128       # m-tiles
NEG_BIG = -1.0e30
DEBUG = False


def build_program(b_loc, nch, n_cores, reps=1):
    S = nch * CS
    TT = 4 * nch                      # 128-row s-tiles per batch
    nc = bacc.Bacc("TRN2", target_bir_lowering=False, debug=False,
                   num_devices=n_cores)

    x_d = nc.dram_tensor("x", [b_loc, S, D], f16, kind="ExternalInput").ap()
    xlT_d = nc.dram_tensor("xlT", [D, b_loc], f32, kind="ExternalInput").ap()
    wq_d = nc.dram_tensor("wq", [D, D], f32, kind="ExternalInput").ap()
    wkT_d = nc.dram_tensor("wkT", [D, D], f32, kind="ExternalInput").ap()
    wv_d = nc.dram_tensor("wv", [D, D], f16, kind="ExternalInput").ap()
    fcT_d = nc.dram_tensor("fcT", [D, D], f16, kind="ExternalInput").ap()
    cvec_d = nc.dram_tensor("cvec", [4, D], f32, kind="ExternalInput").ap()
    mask_d = nc.dram_tensor("mask", [4, 128], f32, kind="ExternalInput").ap()
    eye_d = nc.dram_tensor("eye", [128, 128], f32, kind="ExternalInput").ap()
    out_d = nc.dram_tensor("out", [b_loc, D], f32, kind="ExternalOutput").ap()
    dbg = {}
    if DEBUG:
        HBg = H * b_loc
        dbg["q"] = nc.dram_tensor("dbg_q", [b_loc, D], f32, kind="ExternalOutput").ap()
        dbg["wkq"] = nc.dram_tensor("dbg_wkq", [128, MT, b_loc, 2, H], f32, kind="ExternalOutput").ap()
        dbg["scT"] = nc.dram_tensor("dbg_scT", [128, nch * 128], f32, kind="ExternalOutput").ap()
        dbg["p16"] = nc.dram_tensor("dbg_p16", [128, nch * 128], f32, kind="ExternalOutput").ap()
        dbg["pT"] = nc.dram_tensor("dbg_pT", [128, nch * 128], f32, kind="ExternalOutput").ap()
        dbg["t0"] = nc.dram_tensor("dbg_t0", [16, D], f32, kind="ExternalOutput").ap()
        dbg["ho"] = nc.dram_tensor("dbg_ho", [HBg, D], f32, kind="ExternalOutput").ap()
        dbg["ccT"] = nc.dram_tensor("dbg_ccT", [128, MT, b_loc], f32, kind="ExternalOutput").ap()
        dbg["qblk"] = nc.dram_tensor("dbg_qblk", [128, MT, b_loc * H], f32, kind="ExternalOutput").ap()

    with tile.TileContext(nc) as tc:
      with ExitStack() as top:
        const = top.enter_context(tc.tile_pool(name="const", bufs=1))
        xp = top.enter_context(tc.tile_pool(name="xp", bufs=2))

        ident32 = const.tile([128, 128], f32)
        ident16 = const.tile([128, 128], f16)
        mask_sb = const.tile([128, 128], f32)
        wkq_sb = const.tile([128, MT, b_loc, 3 * H], f16)
        ctxT_all = const.tile([128, MT, H, b_loc], f16)
        eps_sb = const.tile([b_loc, 1], f32)

        nc.vector.memset(eps_sb, 1e-5)

        # ---------------- prologue ----------------
        with tc.tile_pool(name="pro_sb", bufs=1) as pro, \
             tc.tile_pool(name="pro_w", bufs=2) as prow, \
             tc.tile_pool(name="pro_ps", bufs=1, space="PSUM") as pps, \
             tc.tile_pool(name="pro_psq", bufs=1, space="PSUM") as ppsq, \
             tc.tile_pool(name="pro_ps8", bufs=2, space="PSUM") as pps8:
            eye0 = pro.tile([128, 128], f32)
            nc.sync.dma_start(out=eye0, in_=eye_d)
            for j in range(4):
                nc.sync.dma_start(out=mask_sb[32 * j:32 * j + 32, :],
                                  in_=mask_d[j:j + 1, :].to_broadcast((32, 128)))
            nc.vector.tensor_copy(ident32, eye0)
            nc.vector.tensor_copy(ident16, eye0)

            xlT_sb = pro.tile([128, MT, b_loc], f32)
            nc.sync.dma_start(out=xlT_sb, in_=xlT_d.rearrange("(k p) b -> p k b", p=128))

            # PE touch to absorb DVE sem before first transpose
            tch = pps.tile([16, 128], f32, tag="touch")
            nc.tensor.transpose(tch, ident32[:, 0:16], ident32)

            # q = xlast @ Wq / 8, fp32 exact; wq streamed in 4 pieces
            q_ps = ppsq.tile([b_loc, D], f32, tag="q")
            for piece in range(4):
                wqh = prow.tile([128, 2, D], f32, tag="wq")
                nc.sync.dma_start(
                    out=wqh,
                    in_=wq_d[piece * 256:(piece + 1) * 256, :]
                        .rearrange("(k p) n -> p k n", p=128))
                for kl in range(2):
                    k = piece * 2 + kl
                    for hf in range(2):
                        nc.tensor.matmul(q_ps[:, hf * 512:(hf + 1) * 512],
                                         xlT_sb[:, k, :],
                                         wqh[:, kl, hf * 512:(hf + 1) * 512],
                                         start=(k == 0), stop=(k == MT - 1))
            q_sb = pro.tile([b_loc, D], f32)
            nc.scalar.activation(q_sb, q_ps, AF.Copy, scale=0.125)
            qT_sb = pro.tile([128, MT, b_loc], f32)
            qblk = pro.tile([128, MT, b_loc * H], f32)

            for t8 in range(MT):
                trp = pps.tile([128, b_loc], f32, tag="trq")
                nc.tensor.transpose(trp, q_sb[:, t8 * 128:(t8 + 1) * 128],
                                    ident32[0:b_loc, 0:b_loc])
                nc.vector.tensor_copy(qT_sb[:, t8, :], trp)

            # qblk[p, t, b*16+h] = q[b, 128t+p] if h == (128t+p)//64 else 0
            nc.gpsimd.memset(qblk, 0.0)
            for t8 in range(MT):
                for b in range(b_loc):
                    nc.gpsimd.tensor_copy(
                        qblk[0:64, t8, b * H + 2 * t8:b * H + 2 * t8 + 1],
                        qT_sb[0:64, t8, b:b + 1])
                    nc.gpsimd.tensor_copy(
                        qblk[64:128, t8, b * H + 2 * t8 + 1:b * H + 2 * t8 + 2],
                        qT_sb[64:128, t8, b:b + 1])

            # wkq = blockdiag(q) proj of Wk (fp32 exact), then hi/lo fp16 split.
            # k-outer streaming: 8 persistent PSUM accumulators (one per mo).
            bh = b_loc * H
            wkq_ps0 = pps8.tile([128, 512], f32, tag="wkq")
            wkq_ps1 = pps8.tile([128, 512], f32, tag="wkq")
            wkq_ps = [(wkq_ps0 if mo < 4 else wkq_ps1)
                      [:, (mo % 4) * bh:(mo % 4 + 1) * bh] for mo in range(MT)]
            for piece in range(4):
                wkh = prow.tile([128, 2, D], f32, tag="wk")
                nc.sync.dma_start(
                    out=wkh,
                    in_=wkT_d[piece * 256:(piece + 1) * 256, :]
                        .rearrange("(k p) n -> p k n", p=128))
                for kl in range(2):
                    k = piece * 2 + kl
                    for mo in range(MT):
                        nc.tensor.matmul(wkq_ps[mo],
                                         wkh[:, kl, mo * 128:(mo + 1) * 128],
                                         qblk[:, k, :],
                                         start=(k == 0 and mo % 4 == 0),
                                         stop=(k == MT - 1),
                                         skip_group_check=True)
            if DEBUG:
                nc.sync.dma_start(out=dbg["q"], in_=q_sb)
                nc.sync.dma_start(out=dbg["qblk"], in_=qblk)
            nc.gpsimd.memset(wkq_sb, 0.0)
            for mo in range(MT):
                # hi = fp16(wkq); lo = fp16(wkq - hi). wkq_ps cols = b*16+h
                hi = wkq_sb[:, mo, :, 0:H]
                nc.scalar.copy(hi, wkq_ps[mo])
                nc.vector.tensor_tensor(wkq_sb[:, mo, :, 2 * H:3 * H],
                                        wkq_ps[mo], hi, op=OP.subtract)

        if DEBUG:
            nc.gpsimd.dma_start(out=dbg["wkq"], in_=wkq_sb)

        # ---------------- weights for endgame ----------------
        with tc.tile_pool(name="wts", bufs=1) as wts:
          wv_sb = wts.tile([128, MT, D], f16)
          fcT_sb = wts.tile([128, MT, D], f16)
          nc.sync.dma_start(out=wv_sb, in_=wv_d.rearrange("(k p) n -> p k n", p=128))
          nc.sync.dma_start(out=fcT_sb, in_=fcT_d.rearrange("(k p) n -> p k n", p=128))

          with tc.tile_pool(name="x_small", bufs=2) as sm_pool, \
               tc.tile_pool(name="xt_pool", bufs=1) as xt_pool, \
               tc.tile_pool(name="sc_pool", bufs=2) as sc_pool, \
               tc.tile_pool(name="p_pool", bufs=2) as p_pool, \
               tc.tile_pool(name="pt_pool", bufs=1) as pt_pool, \
               tc.tile_pool(name="bt_pool", bufs=1) as bt_pool, \
               tc.tile_pool(name="ps_stage", bufs=2, space="PSUM") as ps_stage, \
               tc.tile_pool(name="ps_sc", bufs=2, space="PSUM") as ps_sc, \
               tc.tile_pool(name="ps_small", bufs=2, space="PSUM") as ps_small, \
               tc.tile_pool(name="ps_ctx", bufs=1, space="PSUM") as ps_ctx:

            for rep in range(reps):
              for b in range(b_loc):
                xb = xp.tile([128, TT, D], f16, tag="x")
                for piece in range((nch + 1) // 2):
                    r0 = piece * 1024
                    r1 = min(S, r0 + 1024)
                    nc.scalar.dma_start(
                        out=xb[:, 8 * piece:8 * piece + (r1 - r0) // 128, :],
                        in_=x_d[b, r0:r1, :].rearrange("(t p) m -> p t m", p=128))

                scT = sc_pool.tile([128, nch * 128], f32, tag="scT")
                nc.vector.memset(scT, 0.0)
                # -------- pass 1: transposes + col-tiled scores --------
                for c in range(nch):
                    xt = xt_pool.tile([128, MT, CS], f16, tag="xt")
                    for mp in range(MT // 2):
                        stg = ps_stage.tile([128, 2, CS], f16, tag="stage")
                        for q in range(2):
                            mt = 2 * mp + q
                            for t in range(4):
                                nc.tensor.transpose(
                                    stg[:, q, t * 128:(t + 1) * 128],
                                    xb[:, 4 * c + t, mt * 128:(mt + 1) * 128],
                                    ident16)
                        if mp % 2 == 0:
                            nc.scalar.copy(xt[:, 2 * mp:2 * mp + 2, :], stg)
                        else:
                            nc.vector.tensor_copy(xt[:, 2 * mp:2 * mp + 2, :], stg)
                    scbt = ps_sc.tile([64, 512], f32, tag="sc")
                    scb = scbt[0:48, :]
                    for mt in range(MT):
                        nc.tensor.matmul(
                            scb, wkq_sb[:, mt, b, :], xt[:, mt, :],
                            start=(mt == 0), stop=(mt == MT - 1),
                            skip_group_check=True)
                    # hi rows 0:16, pad 16:32, lo rows 32:48 (32-aligned)
                    lo_sb = bt_pool.tile([16, 512], f32, tag="losb")
                    nc.scalar.copy(lo_sb, scbt[32:48, :])
                    for j in range(4):
                        dst = scT[32 * j:32 * j + H, c * 128:(c + 1) * 128]
                        nc.vector.tensor_tensor(
                            dst, scbt[0:16, 128 * j:128 * (j + 1)],
                            lo_sb[:, 128 * j:128 * (j + 1)], op=OP.add)
                    if c == nch - 1:
                        for j in range(4):
                            dst = scT[32 * j:32 * j + H, c * 128:(c + 1) * 128]
                            nc.vector.tensor_tensor(
                                dst, dst, mask_sb[32 * j:32 * j + H, :],
                                op=OP.add)

                if DEBUG and rep == 0 and b == 0:
                    nc.sync.dma_start(out=dbg["scT"], in_=scT)
                # -------- batched softmax --------
                mx = sm_pool.tile([128, 1], f32, tag="mx")
                nc.vector.tensor_reduce(mx, scT, axis=AX.X, op=OP.max)
                smt1 = ps_small.tile([128, 128], f32, tag="sm")
                mrow_ps = smt1[0:1, :]
                nc.tensor.transpose(mrow_ps, mx, ident32)
                row = sm_pool.tile([1, 224], f32, tag="rowsb")
                nc.vector.tensor_copy(row[:, 0:128], mrow_ps)
                nc.vector.tensor_tensor(row[:, 128:160], row[:, 0:32],
                                        row[:, 32:64], op=OP.max)
                nc.vector.tensor_tensor(row[:, 160:192], row[:, 64:96],
                                        row[:, 96:128], op=OP.max)
                nc.vector.tensor_tensor(row[:, 192:224], row[:, 128:160],
                                        row[:, 160:192], op=OP.max)
                nrow = sm_pool.tile([1, 128], f32, tag="nrow")
                for j in range(4):
                    nc.vector.tensor_scalar_mul(nrow[:, 32 * j:32 * j + 32],
                                                row[:, 192:224], -1.0)
                smt2 = ps_small.tile([128, 128], f32, tag="sm")
                negm_ps = smt2[:, 0:1]
                nc.tensor.transpose(negm_ps, nrow, ident32[0:1, 0:1])
                negm = sm_pool.tile([128, 1], f32, tag="negm")
                nc.vector.tensor_copy(negm, negm_ps)

                p16 = p_pool.tile([128, nch * 128], f16, tag="p")
                lc = sm_pool.tile([128, 1], f32, tag="lc")
                nc.scalar.activation(p16, scT, AF.Exp, bias=negm, scale=1.0,
                                     accum_out=lc)

                smt3 = ps_small.tile([128, 128], f32, tag="sm")
                lrow_ps = smt3[0:1, :]
                nc.tensor.transpose(lrow_ps, lc, ident32)
                lrow = sm_pool.tile([1, 224], f32, tag="rowsb")
                nc.vector.tensor_copy(lrow[:, 0:128], lrow_ps)
                nc.vector.tensor_tensor(lrow[:, 128:160], lrow[:, 0:32],
                                        lrow[:, 32:64], op=OP.add)
                nc.vector.tensor_tensor(lrow[:, 160:192], lrow[:, 64:96],
                                        lrow[:, 96:128], op=OP.add)
                nc.vector.tensor_tensor(lrow[:, 192:224], lrow[:, 128:160],
                                        lrow[:, 160:192], op=OP.add)
                rrow = sm_pool.tile([1, 128], f32, tag="nrow")
                for j in range(4):
                    nc.vector.reciprocal(rrow[:, 32 * j:32 * j + 32],
                                         lrow[:, 192:224])
                smt4 = ps_small.tile([128, 128], f32, tag="sm")
                rl_ps = smt4[:, 0:1]
                nc.tensor.transpose(rl_ps, rrow, ident32[0:1, 0:1])
                rl_sb = sm_pool.tile([128, 1], f32, tag="rl")
                nc.vector.tensor_copy(rl_sb, rl_ps)
                nc.vector.tensor_scalar(out=p16, in0=p16, scalar1=rl_sb,
                                        scalar2=None, op0=OP.mult)

                if DEBUG and rep == 0 and b == 0:
                    nc.gpsimd.dma_start(out=dbg["p16"], in_=p16)
                pT = pt_pool.tile([128, nch * 128], f16, tag="pT")
                for c in range(nch):
                    ptt = ps_small.tile([128, 128], f32, tag="sm")
                    ptp = ptt[:, 0:64].bitcast(f16)
                    nc.tensor.transpose(ptp, p16[:, c * 128:(c + 1) * 128], ident16)
                    if c % 2 == 0:
                        nc.scalar.copy(pT[:, c * 128:(c + 1) * 128], ptp)
                    else:
                        nc.vector.tensor_copy(pT[:, c * 128:(c + 1) * 128], ptp)

                if DEBUG and rep == 0 and b == 0:
                    nc.gpsimd.dma_start(out=dbg["pT"], in_=pT)
                # -------- pass 2: col-tiled ctx over the whole batch --------
                ctxp = ps_ctx.tile([128, D], f32, tag="ctx")
                for c in range(nch):
                    for hf in range(2):
                        for j in range(4):
                            nc.tensor.matmul(
                                ctxp[32 * j:32 * j + H, hf * 512:(hf + 1) * 512],
                                pT[:, c * 128 + 32 * j:c * 128 + 32 * j + H],
                                xb[:, 4 * c + j, hf * 512:(hf + 1) * 512],
                                start=(c == 0),
                                stop=(c == nch - 1),
                                tile_position=(0, 32 * j),
                                skip_group_check=True)

                t0 = bt_pool.tile([16, D], f32, tag="acc")
                nc.scalar.copy(t0, ctxp[96:112, :])
                nc.vector.tensor_tensor(t0, t0, ctxp[64:80, :], op=OP.add)
                nc.vector.tensor_tensor(t0, t0, ctxp[32:48, :], op=OP.add)
                nc.vector.tensor_tensor(t0, t0, ctxp[0:16, :], op=OP.add)

                if DEBUG and rep == 0 and b == 0:
                    nc.sync.dma_start(out=dbg["t0"], in_=t0)
                for k in range(MT):
                    ctt = ps_small.tile([128, 128], f32, tag="sm")
                    ctp = ctt[:, 0:H]
                    nc.tensor.transpose(ctp, t0[:, k * 128:(k + 1) * 128],
                                        ident32[0:H, 0:H])
                    nc.scalar.copy(ctxT_all[:, k, :, b], ctp)

          # ---------------- endgame ----------------
          with tc.tile_pool(name="end_sb", bufs=1) as end, \
               tc.tile_pool(name="end_ps", bufs=1, space="PSUM") as eps_pool, \
               tc.tile_pool(name="end_ps2", bufs=1, space="PSUM") as eps2, \
               tc.tile_pool(name="end_ps3", bufs=1, space="PSUM") as eps3:
              bias_sb = end.tile([16, D], f32)
              gamma_sb = end.tile([16, D], f32)
              beta_sb = end.tile([16, D], f32)
              nc.sync.dma_start(out=bias_sb, in_=cvec_d[0:1, :].to_broadcast((16, D)))
              nc.sync.dma_start(out=gamma_sb, in_=cvec_d[1:2, :].to_broadcast((16, D)))
              nc.sync.dma_start(out=beta_sb, in_=cvec_d[2:3, :].to_broadcast((16, D)))
              HB = H * b_loc
              ho_ps = eps_pool.tile([HB, D], f32, tag="ho")
              for k in range(MT):
                  for hf in range(2):
                      nc.tensor.matmul(ho_ps[:, hf * 512:(hf + 1) * 512],
                                       ctxT_all[:, k, :, :],
                                       wv_sb[:, k, hf * 512:(hf + 1) * 512],
                                       start=(k == 0), stop=(k == MT - 1))
              ho_sb = end.tile([HB, D], f16)
              nc.vector.tensor_copy(ho_sb, ho_ps)
              # hoT per 128-col block k: [c, (h*b_loc+b)]; diag blocks:
              #   rows 0:64   head 2k   -> cols 2k*b_loc..(2k+1)*b_loc
              #   rows 64:128 head 2k+1 -> cols (2k+1)*b_loc..(2k+2)*b_loc
              if DEBUG:
                  nc.gpsimd.dma_start(out=dbg["ho"], in_=ho_sb)
              ccT = end.tile([128, MT, b_loc], f16)
              for k in range(MT):
                  hot = eps2.tile([128, HB], f16, tag="hot")
                  nc.tensor.transpose(hot, ho_sb[:, k * 128:(k + 1) * 128],
                                      ident16[0:HB, 0:HB])
                  nc.scalar.copy(ccT[0:64, k, :],
                                 hot[0:64, 2 * k * b_loc:(2 * k + 1) * b_loc])
                  nc.scalar.copy(ccT[64:128, k, :],
                                 hot[64:128, (2 * k + 1) * b_loc:(2 * k + 2) * b_loc])

              if DEBUG:
                  nc.gpsimd.dma_start(out=dbg["ccT"], in_=ccT)
              int_ps = eps3.tile([b_loc, D], f32, tag="int")
              for k in range(MT):
                  for hf in range(2):
                      nc.tensor.matmul(int_ps[:, hf * 512:(hf + 1) * 512],
                                       ccT[:, k, :],
                                       fcT_sb[:, k, hf * 512:(hf + 1) * 512],
                                       start=(k == 0), stop=(k == MT - 1))

              int_sb = end.tile([b_loc, D], f32)
              nc.vector.tensor_tensor(int_sb, int_ps, bias_sb[0:b_loc, :],
                                      op=OP.add)
              stats = end.tile([b_loc, 2, 6], f32)
              for g in range(2):
                  nc.vector.bn_stats(stats[:, g, :],
                                     int_sb[:, g * 512:(g + 1) * 512])
              mv = end.tile([b_loc, 2], f32)
              nc.vector.bn_aggr(mv, stats)
              negmean = end.tile([b_loc, 1], f32)
              nc.vector.tensor_scalar_mul(negmean, mv[:, 0:1], -1.0)
              std = end.tile([b_loc, 1], f32)
              nc.scalar.activation(std, mv[:, 1:2], AF.Sqrt, bias=eps_sb,
                                   scale=1.0)
              rstd = end.tile([b_loc, 1], f32)
              nc.vector.reciprocal(rstd, std)
              norm_sb = end.tile([b_loc, D], f32)
              nc.vector.tensor_scalar(out=norm_sb, in0=int_sb, scalar1=negmean,
                                      scalar2=rstd, op0=OP.add, op1=OP.mult)
              nc.vector.tensor_tensor(norm_sb, norm_sb, gamma_sb[0:b_loc, :],
                                      op=OP.mult)
              out_sb = end.tile([b_loc, D], f32)
              nc.vector.tensor_tensor(out_sb, norm_sb, beta_sb[0:b_loc, :],
                                      op=OP.add)
              nc.sync.dma_start(out=out_d, in_=out_sb)

    nc.compile()
    return nc


_prog_cache = {}


def _get_program(b_loc, nch, n_cores, reps=1):
    key = (b_loc, nch, n_cores, reps)
    if key not in _prog_cache:
        _prog_cache[key] = build_program(b_loc, nch, n_cores, reps=reps)
    return _prog_cache[key]


def make_in_maps(data_input, weight_q, weight_k, weight_v, fc_weight, fc_bias,
                 ln_gamma, ln_beta, index, n_cores=8):
    data_input = np.asarray(data_input, dtype=np.float32)
    weight_q = np.asarray(weight_q, dtype=np.float32)
    weight_k = np.asarray(weight_k, dtype=np.float32)
    weight_v = np.asarray(weight_v, dtype=np.float32)
    fc_weight = np.asarray(fc_weight, dtype=np.float32)
    fc_bias = np.asarray(fc_bias, dtype=np.float32)
    ln_gamma = np.asarray(ln_gamma, dtype=np.float32)
    ln_beta = np.asarray(ln_beta, dtype=np.float32)
    idx = int(index)

    B, S_max, _ = data_input.shape
    b_loc = B // n_cores
    s_eff = idx + 1
    nch = max(1, (s_eff + CS - 1) // CS)
    S = nch * CS

    xlT = np.ascontiguousarray(data_input[:, idx, :].T)
    wkT = np.ascontiguousarray(weight_k.T)
    wv16 = weight_v.astype(np.float16)
    fcT16 = np.ascontiguousarray(fc_weight.T).astype(np.float16)
    cvec = np.zeros((4, D), np.float32)
    cvec[0] = fc_bias
    cvec[1] = ln_gamma
    cvec[2] = ln_beta
    mask4 = np.zeros((4, 128), np.float32)
    tail = s_eff - (nch - 1) * CS
    for j in range(4):
        for s in range(128):
            if j * 128 + s >= tail:
                mask4[j, s] = NEG_BIG
    eye = np.eye(128, dtype=np.float32)

    in_maps = []
    for core in range(n_cores):
        b0 = core * b_loc
        xc = np.ascontiguousarray(data_input[b0:b0 + b_loc, :S, :]).astype(np.float16)
        in_maps.append({
            "x": xc,
            "xlT": np.ascontiguousarray(xlT[:, b0:b0 + b_loc]),
            "wq": weight_q, "wkT": wkT, "wv": wv16, "fcT": fcT16,
            "cvec": cvec, "mask": mask4, "eye": eye,
        })
    return in_maps, b_loc, nch, B


def kernel(data_input, weight_q, weight_k, weight_v, fc_weight, fc_bias,
           ln_gamma, ln_beta, index):
    n_cores = 8
    in_maps, b_loc, nch, B = make_in_maps(
        data_input, weight_q, weight_k, weight_v, fc_weight, fc_bias,
        ln_gamma, ln_beta, index, n_cores)
    nc = _get_program(b_loc, nch, n_cores)

    kw = {}
    if os.environ.get("KERNEL_TRACE"):
        kw["trace"] = True
        td = os.environ.get("KERNEL_TRACE_DIR")
        if td:
            os.makedirs(td, exist_ok=True)
            kw["tmpdir"] = td
    try:
        res = bass_utils.run_bass_kernel_spmd(
            nc, in_maps, core_ids=list(range(n_cores)), **kw)
    except ModuleNotFoundError:
        res = bass_utils.run_bass_kernel_spmd(
            nc, in_maps, core_ids=list(range(n_cores)))
    global LAST_EXEC_NS, LAST_RESULTS
    if getattr(res, "exec_time_ns", None) is not None:
        LAST_EXEC_NS = res.exec_time_ns
        LAST_RESULTS = res
    out = np.concatenate([res.results[c]["out"] for c in range(n_cores)], axis=0)
    return out.reshape(B, 1, D).astype(np.float32)


LAST_EXEC_NS = None
LAST_RESULTS = None

